# revision 7
# baseline (speedup 1.0000x reference)
"""AGThenGemm, data-parallel: shard B across 8 cores, replicate weights, NO collectives.

Rationale (measured): arming the collective hardware permanently downshifts the PE
clock ~20% for the whole NEFF (225 -> 270 ns per 512-row matmul even for a 1KB
AllGather). Data-parallel does the same 43 GFLOP/core with zero collectives and
streams weights (32+128 MB/core) at ~285 GB/s, under the ~358 GB/s per-core share.

Per core r (B_LOCAL = B/8 = 256):
  GEMM1: act_T[D, B_LOCAL] = W_prev^T @ A_local^T   (A pre-transposed on host)
         -> act_T kept RESIDENT in SBUF (1 MB), never touches DRAM.
  GEMM2: out[B_LOCAL, F] = act_T^T @ W_up, streamed over F blocks.
  Inner loops rotate PSUM banks so drains never serialize against streams.
"""

from contextlib import ExitStack

import numpy as np

import concourse.bass as bass
import concourse.tile as tile
from concourse import bacc, mybir
from concourse.bass_utils import run_bass_kernel_spmd

N_CORES = 8
B, K_PREV, D, F = 2048, 4096, 4096, 16384
B_LOCAL = B // N_CORES

P = 128


def build_nc(
    b_local=B_LOCAL,
    k_prev=K_PREV,
    d=D,
    f=F,
    n_cores=N_CORES,
    wp_bufs=8,
    wu_bufs=6,
    interleave_a=True,
    debug=False,
):
    nc = bacc.Bacc(
        "TRN2",
        target_bir_lowering=False,
        debug=debug,
        num_devices=n_cores,
    )
    dt = mybir.dt.float16

    a_t = nc.dram_tensor("a_t", [k_prev, b_local], dt, kind="ExternalInput")
    w_prev = nc.dram_tensor("w_prev", [k_prev, d], dt, kind="ExternalInput")
    w_up = nc.dram_tensor("w_up", [d, f], dt, kind="ExternalInput")
    out = nc.dram_tensor("out", [b_local, f], dt, kind="ExternalOutput")

    a3 = a_t.ap().rearrange("(ko p) n -> p ko n", p=P)  # [P, k_prev/P, b_local]
    wp3 = w_prev.ap().rearrange("(ko p) m -> p ko m", p=P)
    wu3 = w_up.ap().rearrange("(ko p) n -> p ko n", p=P)
    out3 = out.ap().rearrange("(mo p) n -> p mo n", p=P)  # [P, b_local/P, f]

    K1_SUB = k_prev // P  # k-subtiles for GEMM1 contraction
    K2_SUB = d // P
    M1_TILE = min(512, d)  # GEMM1 output-D tile
    M1_SUB = M1_TILE // P
    M1_TILES = d // M1_TILE
    KT1 = k_prev // 512  # 512-wide k tiles
    B_SUB = b_local // P  # GEMM2 output-B subtiles (2)
    NB = min(1024, f)  # GEMM2 F block
    N_SUB = NB // 512
    NBLKS = f // NB
    KT2 = d // 512

    with tile.TileContext(nc) as tc:
        with ExitStack() as ctx:
            wp_pool = ctx.enter_context(tc.tile_pool(name="wp_pool", bufs=wp_bufs))
            wu_pool = ctx.enter_context(tc.tile_pool(name="wu_pool", bufs=wu_bufs))
            temps = ctx.enter_context(tc.tile_pool(name="temps", bufs=3))
            res_pool = ctx.enter_context(tc.tile_pool(name="res_pool", bufs=1))
            psum = ctx.enter_context(tc.tile_pool(name="psum", bufs=2, space="PSUM"))

            a_res = res_pool.tile([P, K1_SUB, b_local], dt, name="a_res")
            act_res = res_pool.tile([P, K2_SUB, b_local], dt, name="act_res")

            # Load A^T chunk 0 up front; the rest interleave with the first
            # W_prev panel loads so the first matmul isn't queued behind 2 MB.
            if interleave_a:
                nc.sync.dma_start(a_res[:, 0:4, :], a3[:, 0:4, :])
            else:
                for j in range(KT1):
                    nc.sync.dma_start(
                        a_res[:, j * 4 : (j + 1) * 4, :],
                        a3[:, j * 4 : (j + 1) * 4, :],
                    )

            # GEMM1: act_T = W_prev^T @ A^T, evicted straight into SBUF act_res.
            for mt in range(M1_TILES):
                ps = [
                    psum.tile([P, 512], mybir.dt.float32, name=f"ps{mi}", tag=f"ps{mi}")[
                        :, :b_local
                    ]
                    for mi in range(M1_SUB)
                ]
                for kt in range(KT1):
                    wp_t = wp_pool.tile([P, 4, M1_TILE], dt, name="wp_t", tag="wp_t")
                    nc.sync.dma_start(
                        wp_t[:],
                        wp3[:, kt * 4 : (kt + 1) * 4,
                            mt * M1_TILE : (mt + 1) * M1_TILE],
                    )
                    if interleave_a and mt == 0 and kt + 1 < KT1:
                        j = kt + 1
                        nc.sync.dma_start(
                            a_res[:, j * 4 : (j + 1) * 4, :],
                            a3[:, j * 4 : (j + 1) * 4, :],
                        )
                    for ki in range(4):
                        for mi in range(M1_SUB):
                            nc.tensor.matmul(
                                ps[mi][:],
                                wp_t[:, ki, mi * P : (mi + 1) * P],
                                a_res[:, kt * 4 + ki, :],
                                start=(kt == 0 and ki == 0),
                                stop=(kt == KT1 - 1 and ki == 3),
                            )
                for mi in range(M1_SUB):
                    nc.vector.tensor_copy(
                        act_res[:, mt * M1_SUB + mi, :], ps[mi][:]
                    )

            # GEMM2: out = act_T^T @ W_up, streaming W_up once.
            for nb in range(NBLKS):
                ps2 = [
                    psum.tile(
                        [P, 512], mybir.dt.float32, name=f"ps{j}", tag=f"ps{j}"
                    )
                    for j in range(B_SUB * N_SUB)
                ]
                for kt in range(KT2):
                    wu_t = wu_pool.tile([P, 4, NB], dt, name="wu_t", tag="wu_t")
                    nc.sync.dma_start(
                        wu_t[:],
                        wu3[:, kt * 4 : (kt + 1) * 4, nb * NB : (nb + 1) * NB],
                    )
                    for ki in range(4):
                        for mi in range(B_SUB):
                            for ni in range(N_SUB):
                                nc.tensor.matmul(
                                    ps2[mi * N_SUB + ni][:],
                                    act_res[:, kt * 4 + ki, mi * P : (mi + 1) * P],
                                    wu_t[:, ki, ni * 512 : (ni + 1) * 512],
                                    start=(kt == 0 and ki == 0),
                                    stop=(kt == KT2 - 1 and ki == 3),
                                )
                ot = temps.tile([P, B_SUB, NB], dt, name="ot", tag="ot")
                for mi in range(B_SUB):
                    for ni in range(N_SUB):
                        nc.vector.tensor_copy(
                            ot[:, mi, ni * 512 : (ni + 1) * 512],
                            ps2[mi * N_SUB + ni][:],
                        )
                nc.sync.dma_start(
                    out3[:, :, nb * NB : (nb + 1) * NB], ot[:]
                )
    nc.compile()
    return nc


_NC_CACHE = {}


def _get_nc():
    if "nc" not in _NC_CACHE:
        _NC_CACHE["nc"] = build_nc()
    return _NC_CACHE["nc"]


def run(A_prev, W_prev, W_up, **spmd_kwargs):
    W_prev = np.ascontiguousarray(W_prev)
    W_up = np.ascontiguousarray(W_up)
    in_maps = []
    for r in range(N_CORES):
        a_loc = np.ascontiguousarray(
            A_prev[r * B_LOCAL : (r + 1) * B_LOCAL, :].T
        )
        in_maps.append({"a_t": a_loc, "w_prev": W_prev, "w_up": W_up})
    nc = _get_nc()
    res = run_bass_kernel_spmd(
        nc, in_maps, core_ids=list(range(N_CORES)), **spmd_kwargs
    )
    out = np.concatenate([res.results[r]["out"] for r in range(N_CORES)], axis=0)
    return out, res


def kernel(A_prev, W_prev, W_up):
    return run(A_prev, W_prev, W_up)[0]


# revision 8
# speedup vs baseline: 1.1168x; 1.1168x over previous
"""AGThenGemm, data-parallel: shard B across 8 cores, replicate weights, NO collectives.

Rationale (measured): arming the collective hardware permanently downshifts the PE
clock ~20% for the whole NEFF (225 -> 270 ns per 512-row matmul even for a 1KB
AllGather). Data-parallel does the same 43 GFLOP/core with zero collectives and
streams weights (32+128 MB/core) at ~285 GB/s, under the ~358 GB/s per-core share.

Per core r (B_LOCAL = B/8 = 256):
  GEMM1: act_T[D, B_LOCAL] = W_prev^T @ A_local^T   (A pre-transposed on host)
         -> act_T kept RESIDENT in SBUF (1 MB), never touches DRAM.
  GEMM2: out[B_LOCAL, F] = act_T^T @ W_up, streamed over F blocks.
  Inner loops rotate PSUM banks so drains never serialize against streams.
"""

from contextlib import ExitStack

import numpy as np

import concourse.tile as tile
from concourse import bacc, mybir
from concourse.bass_utils import run_bass_kernel_spmd

N_CORES = 8
B, K_PREV, D, F = 2048, 4096, 4096, 16384
B_LOCAL = B // N_CORES

P = 128


def build_nc(
    b_local=B_LOCAL,
    k_prev=K_PREV,
    d=D,
    f=F,
    n_cores=N_CORES,
    wp_bufs=8,
    wu_bufs=6,
    interleave_a=True,
    debug=False,
):
    nc = bacc.Bacc(
        "TRN2",
        target_bir_lowering=False,
        debug=debug,
        num_devices=n_cores,
    )
    dt = mybir.dt.float16

    a_t = nc.dram_tensor("a_t", [k_prev, b_local], dt, kind="ExternalInput")
    w_prev = nc.dram_tensor("w_prev", [k_prev, d], dt, kind="ExternalInput")
    w_up = nc.dram_tensor("w_up", [d, f], dt, kind="ExternalInput")
    out = nc.dram_tensor("out", [b_local, f], dt, kind="ExternalOutput")

    a3 = a_t.ap().rearrange("(ko p) n -> p ko n", p=P)  # [P, k_prev/P, b_local]
    wp3 = w_prev.ap().rearrange("(ko p) m -> p ko m", p=P)
    wu3 = w_up.ap().rearrange("(ko p) n -> p ko n", p=P)
    out3 = out.ap().rearrange("(mo p) n -> p mo n", p=P)  # [P, b_local/P, f]

    K1_SUB = k_prev // P  # k-subtiles for GEMM1 contraction
    K2_SUB = d // P
    M1_TILE = min(512, d)  # GEMM1 output-D tile
    M1_SUB = M1_TILE // P
    M1_TILES = d // M1_TILE
    KT1 = k_prev // 512  # 512-wide k tiles
    B_SUB = b_local // P  # GEMM2 output-B subtiles (2)
    NB = min(1024, f)  # GEMM2 F block
    N_SUB = NB // 512
    NBLKS = f // NB
    KT2 = d // 512

    with tile.TileContext(nc) as tc:
        with ExitStack() as ctx:
            wp_pool = ctx.enter_context(tc.tile_pool(name="wp_pool", bufs=wp_bufs))
            wu_pool = ctx.enter_context(tc.tile_pool(name="wu_pool", bufs=wu_bufs))
            temps = ctx.enter_context(tc.tile_pool(name="temps", bufs=3))
            res_pool = ctx.enter_context(tc.tile_pool(name="res_pool", bufs=1))
            psum = ctx.enter_context(tc.tile_pool(name="psum", bufs=2, space="PSUM"))

            a_res = res_pool.tile([P, K1_SUB, b_local], dt, name="a_res")
            act_res = res_pool.tile([P, K2_SUB, b_local], dt, name="act_res")

            # Load A^T chunk 0 up front; the rest interleave with the first
            # W_prev panel loads so the first matmul isn't queued behind 2 MB.
            if interleave_a:
                nc.sync.dma_start(a_res[:, 0:4, :], a3[:, 0:4, :])
            else:
                for j in range(KT1):
                    nc.sync.dma_start(
                        a_res[:, j * 4 : (j + 1) * 4, :],
                        a3[:, j * 4 : (j + 1) * 4, :],
                    )

            # GEMM1: act_T = W_prev^T @ A^T, evicted straight into SBUF act_res.
            for mt in range(M1_TILES):
                ps = [
                    psum.tile([P, 512], mybir.dt.float32, name=f"ps{mi}", tag=f"ps{mi}")[
                        :, :b_local
                    ]
                    for mi in range(M1_SUB)
                ]
                for kt in range(KT1):
                    wp_t = wp_pool.tile([P, 4, M1_TILE], dt, name="wp_t", tag="wp_t")
                    nc.sync.dma_start(
                        wp_t[:],
                        wp3[:, kt * 4 : (kt + 1) * 4,
                            mt * M1_TILE : (mt + 1) * M1_TILE],
                    )
                    if interleave_a and mt == 0 and kt + 1 < KT1:
                        j = kt + 1
                        nc.sync.dma_start(
                            a_res[:, j * 4 : (j + 1) * 4, :],
                            a3[:, j * 4 : (j + 1) * 4, :],
                        )
                    for ki in range(4):
                        for mi in range(M1_SUB):
                            nc.tensor.matmul(
                                ps[mi][:],
                                wp_t[:, ki, mi * P : (mi + 1) * P],
                                a_res[:, kt * 4 + ki, :],
                                start=(kt == 0 and ki == 0),
                                stop=(kt == KT1 - 1 and ki == 3),
                            )
                for mi in range(M1_SUB):
                    nc.vector.tensor_copy(
                        act_res[:, mt * M1_SUB + mi, :], ps[mi][:]
                    )

            # GEMM2: out = act_T^T @ W_up, streaming W_up once.
            for nb in range(NBLKS):
                ps2 = [
                    psum.tile(
                        [P, 512], mybir.dt.float32, name=f"ps{j}", tag=f"ps{j}"
                    )
                    for j in range(B_SUB * N_SUB)
                ]
                for kt in range(KT2):
                    wu_t = wu_pool.tile([P, 4, NB], dt, name="wu_t", tag="wu_t")
                    nc.sync.dma_start(
                        wu_t[:],
                        wu3[:, kt * 4 : (kt + 1) * 4, nb * NB : (nb + 1) * NB],
                    )
                    for ki in range(4):
                        for mi in range(B_SUB):
                            for ni in range(N_SUB):
                                nc.tensor.matmul(
                                    ps2[mi * N_SUB + ni][:],
                                    act_res[:, kt * 4 + ki, mi * P : (mi + 1) * P],
                                    wu_t[:, ki, ni * 512 : (ni + 1) * 512],
                                    start=(kt == 0 and ki == 0),
                                    stop=(kt == KT2 - 1 and ki == 3),
                                )
                ot = temps.tile([P, B_SUB, NB], dt, name="ot", tag="ot")
                for mi in range(B_SUB):
                    for ni in range(N_SUB):
                        nc.vector.tensor_copy(
                            ot[:, mi, ni * 512 : (ni + 1) * 512],
                            ps2[mi * N_SUB + ni][:],
                        )
                nc.sync.dma_start(
                    out3[:, :, nb * NB : (nb + 1) * NB], ot[:]
                )
    nc.compile()
    return nc


_NC_CACHE = {}


def _get_nc():
    if "nc" not in _NC_CACHE:
        _NC_CACHE["nc"] = build_nc()
    return _NC_CACHE["nc"]


def run(A_prev, W_prev, W_up, **spmd_kwargs):
    W_prev = np.ascontiguousarray(W_prev)
    W_up = np.ascontiguousarray(W_up)
    in_maps = []
    for r in range(N_CORES):
        a_loc = np.ascontiguousarray(
            A_prev[r * B_LOCAL : (r + 1) * B_LOCAL, :].T
        )
        in_maps.append({"a_t": a_loc, "w_prev": W_prev, "w_up": W_up})
    nc = _get_nc()
    res = run_bass_kernel_spmd(
        nc, in_maps, core_ids=list(range(N_CORES)), **spmd_kwargs
    )
    out = np.concatenate([res.results[r]["out"] for r in range(N_CORES)], axis=0)
    return out, res


def kernel(A_prev, W_prev, W_up):
    return run(A_prev, W_prev, W_up)[0]


# revision 9
# speedup vs baseline: 1.1175x; 1.0007x over previous
"""AGThenGemm, data-parallel: shard B across 8 cores, replicate weights, NO collectives.

Rationale (measured): arming the collective hardware permanently downshifts the PE
clock ~20% for the whole NEFF (225 -> 270 ns per 512-row matmul even for a 1KB
AllGather). Data-parallel does the same 43 GFLOP/core with zero collectives and
streams weights (32+128 MB/core) at ~285 GB/s, under the ~358 GB/s per-core share.

Per core r (B_LOCAL = B/8 = 256):
  GEMM1: act_T[D, B_LOCAL] = W_prev^T @ A_local^T   (A pre-transposed on host)
         -> act_T kept RESIDENT in SBUF (1 MB), never touches DRAM.
  GEMM2: out[B_LOCAL, F] = act_T^T @ W_up, streamed over F blocks.
  Inner loops rotate PSUM banks so drains never serialize against streams.
"""

from contextlib import ExitStack

import numpy as np

import concourse.tile as tile
from concourse import bacc, mybir
from concourse.bass_utils import run_bass_kernel_spmd

N_CORES = 8
B, K_PREV, D, F = 2048, 4096, 4096, 16384
B_LOCAL = B // N_CORES

P = 128


def build_nc(
    b_local=B_LOCAL,
    k_prev=K_PREV,
    d=D,
    f=F,
    n_cores=N_CORES,
    wp_bufs=8,
    wu_bufs=6,
    interleave_a=True,
    debug=False,
):
    nc = bacc.Bacc(
        "TRN2",
        target_bir_lowering=False,
        debug=debug,
        num_devices=n_cores,
    )
    dt = mybir.dt.float16

    a_t = nc.dram_tensor("a_t", [k_prev, b_local], dt, kind="ExternalInput")
    w_prev = nc.dram_tensor("w_prev", [k_prev, d], dt, kind="ExternalInput")
    w_up = nc.dram_tensor("w_up", [d, f], dt, kind="ExternalInput")
    out = nc.dram_tensor("out", [b_local, f], dt, kind="ExternalOutput")

    a3 = a_t.ap().rearrange("(ko p) n -> p ko n", p=P)  # [P, k_prev/P, b_local]
    wp3 = w_prev.ap().rearrange("(ko p) m -> p ko m", p=P)
    wu3 = w_up.ap().rearrange("(ko p) n -> p ko n", p=P)
    out3 = out.ap().rearrange("(mo p) n -> p mo n", p=P)  # [P, b_local/P, f]

    K1_SUB = k_prev // P  # k-subtiles for GEMM1 contraction
    K2_SUB = d // P
    M1_TILE = min(512, d)  # GEMM1 output-D tile
    M1_SUB = M1_TILE // P
    M1_TILES = d // M1_TILE
    KT1 = k_prev // 512  # 512-wide k tiles
    B_SUB = b_local // P  # GEMM2 output-B subtiles (2)
    NB = min(1024, f)  # GEMM2 F block
    N_SUB = NB // 512
    NBLKS = f // NB
    KT2 = d // 512

    with tile.TileContext(nc) as tc:
        with ExitStack() as ctx:
            wp_pool = ctx.enter_context(tc.tile_pool(name="wp_pool", bufs=wp_bufs))
            wu_pool = ctx.enter_context(tc.tile_pool(name="wu_pool", bufs=wu_bufs))
            temps = ctx.enter_context(tc.tile_pool(name="temps", bufs=3))
            res_pool = ctx.enter_context(tc.tile_pool(name="res_pool", bufs=1))
            psum = ctx.enter_context(tc.tile_pool(name="psum", bufs=2, space="PSUM"))

            a_res = res_pool.tile([P, K1_SUB, b_local], dt, name="a_res")
            act_res = res_pool.tile([P, K2_SUB, b_local], dt, name="act_res")

            # Load A^T chunk 0 up front; the rest interleave with the first
            # W_prev panel loads so the first matmul isn't queued behind 2 MB.
            if interleave_a:
                nc.sync.dma_start(a_res[:, 0:4, :], a3[:, 0:4, :])
            else:
                for j in range(KT1):
                    nc.sync.dma_start(
                        a_res[:, j * 4 : (j + 1) * 4, :],
                        a3[:, j * 4 : (j + 1) * 4, :],
                    )

            # GEMM1: act_T = W_prev^T @ A^T, evicted straight into SBUF act_res.
            for mt in range(M1_TILES):
                ps = [
                    psum.tile([P, 512], mybir.dt.float32, name=f"ps{mi}", tag=f"ps{mi}")[
                        :, :b_local
                    ]
                    for mi in range(M1_SUB)
                ]
                for kt in range(KT1):
                    wp_t = wp_pool.tile([P, 4, M1_TILE], dt, name="wp_t", tag="wp_t")
                    nc.sync.dma_start(
                        wp_t[:],
                        wp3[:, kt * 4 : (kt + 1) * 4,
                            mt * M1_TILE : (mt + 1) * M1_TILE],
                    )
                    if interleave_a and mt == 0 and kt + 1 < KT1:
                        j = kt + 1
                        nc.sync.dma_start(
                            a_res[:, j * 4 : (j + 1) * 4, :],
                            a3[:, j * 4 : (j + 1) * 4, :],
                        )
                    for ki in range(4):
                        for mi in range(M1_SUB):
                            nc.tensor.matmul(
                                ps[mi][:],
                                wp_t[:, ki, mi * P : (mi + 1) * P],
                                a_res[:, kt * 4 + ki, :],
                                start=(kt == 0 and ki == 0),
                                stop=(kt == KT1 - 1 and ki == 3),
                            )
                for mi in range(M1_SUB):
                    nc.vector.tensor_copy(
                        act_res[:, mt * M1_SUB + mi, :], ps[mi][:]
                    )

            # GEMM2: out = act_T^T @ W_up, streaming W_up once.
            for nb in range(NBLKS):
                ps2 = [
                    psum.tile(
                        [P, 512], mybir.dt.float32, name=f"ps{j}", tag=f"ps{j}"
                    )
                    for j in range(B_SUB * N_SUB)
                ]
                for kt in range(KT2):
                    wu_t = wu_pool.tile([P, 4, NB], dt, name="wu_t", tag="wu_t")
                    nc.sync.dma_start(
                        wu_t[:],
                        wu3[:, kt * 4 : (kt + 1) * 4, nb * NB : (nb + 1) * NB],
                    )
                    for ki in range(4):
                        for mi in range(B_SUB):
                            for ni in range(N_SUB):
                                nc.tensor.matmul(
                                    ps2[mi * N_SUB + ni][:],
                                    act_res[:, kt * 4 + ki, mi * P : (mi + 1) * P],
                                    wu_t[:, ki, ni * 512 : (ni + 1) * 512],
                                    start=(kt == 0 and ki == 0),
                                    stop=(kt == KT2 - 1 and ki == 3),
                                )
                ot = temps.tile([P, B_SUB, NB], dt, name="ot", tag="ot")
                for mi in range(B_SUB):
                    for ni in range(N_SUB):
                        nc.vector.tensor_copy(
                            ot[:, mi, ni * 512 : (ni + 1) * 512],
                            ps2[mi * N_SUB + ni][:],
                        )
                nc.sync.dma_start(
                    out3[:, :, nb * NB : (nb + 1) * NB], ot[:]
                )
    nc.compile()
    return nc


_NC_CACHE = {}


def _get_nc():
    if "nc" not in _NC_CACHE:
        _NC_CACHE["nc"] = build_nc()
    return _NC_CACHE["nc"]


def run(A_prev, W_prev, W_up, **spmd_kwargs):
    A_prev = np.asarray(A_prev, dtype=np.float16)
    W_prev = np.ascontiguousarray(W_prev, dtype=np.float16)
    W_up = np.ascontiguousarray(W_up, dtype=np.float16)
    in_maps = []
    for r in range(N_CORES):
        a_loc = np.ascontiguousarray(
            A_prev[r * B_LOCAL : (r + 1) * B_LOCAL, :].T
        )
        in_maps.append({"a_t": a_loc, "w_prev": W_prev, "w_up": W_up})
    nc = _get_nc()
    res = run_bass_kernel_spmd(
        nc, in_maps, core_ids=list(range(N_CORES)), **spmd_kwargs
    )
    out = np.concatenate([res.results[r]["out"] for r in range(N_CORES)], axis=0)
    return out, res


def kernel(A_prev, W_prev, W_up):
    return run(A_prev, W_prev, W_up)[0]
